# revision 19
# baseline (speedup 1.0000x reference)
"""BioRNN Trainium2 kernel — time-sharded, scaled-basis recurrence.

Sharding: 8 cores split T=1000 into 125-step output slices; each core runs
the FULL batch (64) for S=192 local steps: 67 warmup steps (leak 0.8 forgets
the h=0 init; measured truncation err ~7e-4) + 125 output steps.

Per-core math in a scaled basis (J=32 step blocks, j = t mod J,
W' = 0.2*w_eff, z' = 0.2*(x @ w_in + noise + b_rec)):
    A_j  = p_t / 0.8^j       PSUM f32, accumulate-only: A += r~_{j-1} @ W'
    r~_j = r_t / 0.8^(j+1)   = relu((A_j + Z~_j) * 1.25)   DVE -> fp16 ring
    Z~_j = z'_t / 0.8^j      zbuf (noise part DMA'd pre-scaled, x@w_in added
                             on-device via PE + Pool)
Block boundary: A'_0 = 0.8^J * A_J via ACT mul -> fp16 -> identity-matmul
seed into the opposite PSUM bank pair.  No per-step decay ops remain: a step
is 16 LDW+MM pairs (N=64) + 2 DVE relu ops.  h_t itself is reconstructed on
the host from the dumped r~ stream by a per-block f32 cumsum:
    h_t = 0.8^(j+1) * (h_{blockstart-1} + sum_{i<=j} r~_i).

Host does pure marshalling/cumsum: inputs pre-transposed to hidden-major
fp16 images with the 0.2*0.8^(-j) scale baked in; output is the raw fp16
r~-image.

Layouts (per core, hidden chunk k = r//128, partition p = r%128):
  w16   [128, k*512 + m*128 + c] = W'[k*128+p, m*128+c]      fp16
  win16 [128, 512]   (n_in on partitions)                    fp16
  xT16  [128, tl*64 + b]      = x[b, g0+tl, p] * sc(tl)      fp16
  zbuf  [128, (tl%ZR)*256 + k*64 + b]  ring                  fp16
  rbuf  [128, (tl%RU)*256 + k*64 + b]  ring (r~)             fp16
  out   [128, tl*256 + k*64 + b]  (raw r~ dump, all S steps) fp16
"""

import numpy as np
from contextlib import ExitStack

import concourse.bass as bass
import concourse.mybir as mybir
import concourse.tile as tile
from concourse import bacc
from concourse import dve_ops
from concourse.dve_spec import Spec, Src0, Src1, C0, relu as _dve_relu_expr, lower
from concourse.dve_uop import DveOpSpec
from concourse.masks import make_identity


def _register_relu_add_sc():
    """Register fused out = relu((in0 + in1) * s0) custom DVE op (idempotent)."""
    name = "RELU_ADD_SC_BIO"
    for o in dve_ops.OPS:
        if o.name == name:
            return o
    opcode = max(dve_ops._SUB_OPCODE_FOR_NAME.values()) + 1
    assert opcode < 0x20
    dve_ops._SUB_OPCODE_FOR_NAME[name] = opcode

    def _ref(in0, in1, c0, c1, c2):
        a = in0.astype(np.float32).reshape(in0.shape[0], -1)
        b = in1.astype(np.float32).reshape(in1.shape[0], -1)
        s = np.maximum(np.nan_to_num((a + b) * c0, nan=0.0, posinf=np.inf,
                                     neginf=-np.inf), 0)
        return s.reshape(in0.shape)

    spec = Spec(body=_dve_relu_expr((Src0 + Src1) * C0), reference=_ref)
    shas = {}
    for ver in ("v3", "v4"):
        s = DveOpSpec(name=name, opcode=opcode, uops=lower(spec, ver=ver),
                      rd1_en=True)
        shas[ver] = s.sha(ver)
    op = dve_ops.DveOp(name, spec, subdim=False, uops_sha=shas)
    dve_ops.OPS.append(op)
    dve_ops.CUSTOM_DVE_SPECS[name] = spec
    return op


RELU_ADD_SC = _register_relu_add_sc()

F32 = mybir.dt.float32
F16 = mybir.dt.float16
AOP = mybir.AluOpType

B = 64           # batch (full, replicated across cores)
R = 512          # n_rec
NIN = 128        # n_in
RC = 4           # hidden chunks of 128
SUP = RC * B     # 256 cols per step
N_CORES = 8
ALPHA = 0.2
LEAK = 1.0 - ALPHA

S = 176          # local steps per core
WU_OFF = 51      # g0 = 125*c - WU_OFF
RU = 64          # r~ ring steps
ZR = 48          # zbuf ring steps (3 windows)
WIN = 16         # zmm/noise/dump window
BLOCKS = [16, 40, 40, 40, 40]       # rescale block lengths (sum = S)
_J_OF, _BLK_OF, _LPREV = [], [], {}
for _bi, _L in enumerate(BLOCKS):
    _s = len(_J_OF)
    if _bi > 0:
        _LPREV[_s] = BLOCKS[_bi - 1]
    for _j in range(_L):
        _J_OF.append(_j)
        _BLK_OF.append(_bi)
assert len(_J_OF) == S


def build_nc(T=1000, use_bacc=True):
    assert T == 1000
    nc = bacc.Bacc() if use_bacc else bass.Bass()

    x_d = nc.dram_tensor("x_img", [128, S * B], F16, kind="ExternalInput").ap()
    n_d = nc.dram_tensor("noise_img", [128, S * SUP], F16,
                         kind="ExternalInput").ap()
    w_d = nc.dram_tensor("w16_img", [128, RC * R], F16,
                         kind="ExternalInput").ap()
    wi_d = nc.dram_tensor("win16_img", [128, R], F16,
                          kind="ExternalInput").ap()
    o_d = nc.dram_tensor("out_img", [128, S * SUP], F16,
                         kind="ExternalOutput").ap()

    pe_last = [None]

    def pe_mm(*args, **kwargs):
        mm = nc.tensor.matmul(*args, **kwargs)
        if pe_last[0] is not None:
            tile.add_dep_helper(mm.ins, pe_last[0], sync=False,
                                reason="pe program order")
        pe_last[0] = mm.ins
        return mm

    with tile.TileContext(nc) as tc, ExitStack() as ctx:
        const = ctx.enter_context(tc.tile_pool(name="const", bufs=1))
        big = ctx.enter_context(tc.tile_pool(name="big", bufs=1))

        ident16 = const.tile([128, 128], F16)
        make_identity(nc, ident16[:, :])
        zero16 = const.tile([128, B], F16)
        nc.vector.memset(zero16[:, :], 0.0)

        w16 = const.tile([128, RC * R], F16)
        nc.sync.dma_start(out=w16[:, :], in_=w_d)
        win16 = const.tile([128, R], F16)
        nc.sync.dma_start(out=win16[:, :], in_=wi_d)

        xT16 = big.tile([128, S * B], F16)
        for xc in range(6):
            c0, c1 = xc * 2 * WIN * B, min((xc + 1) * 2 * WIN * B, S * B)
            nc.gpsimd.dma_start(out=xT16[:, c0:c1], in_=x_d[:, c0:c1])

        zbuf = big.tile([128, ZR * SUP], F16)
        rbuf = big.tile([128, RU * SUP], F16)

        zv = zbuf[:, :].rearrange("p (t k b) -> p t k b", k=RC, b=B)

        ps_z = ctx.enter_context(tc.tile_pool(name="psz", bufs=3, space="PSUM"))
        xst_pool = ctx.enter_context(tc.tile_pool(name="xst", bufs=4))

        def emit_noise_dma(w):
            rt0 = (w * WIN) % ZR
            nc.sync.dma_start(
                out=zbuf[:, rt0 * SUP:(rt0 + WIN) * SUP],
                in_=n_d[:, w * WIN * SUP:(w + 1) * WIN * SUP],
            )

        # zmm piece: one (8-step chunk, m, quarter) 128-col matmul; after the
        # 4th quarter, a Pool TT-add folds psum into zbuf (noise already there).
        zmm_state = {}

        def emit_zmm_piece(w, idx):
            c8, m, q = idx // 16, (idx // 4) % 4, idx % 4
            assert c8 < WIN // 8
            t0 = w * WIN + c8 * 8
            if q == 0:
                zmm_state[(w, c8, m)] = ps_z.tile([128, 8 * B], F32,
                                                  name="zps", tag="zps")
            zps = zmm_state[(w, c8, m)]
            pe_mm(
                zps[:, q * 128:(q + 1) * 128],
                lhsT=win16[:, m * 128:(m + 1) * 128],
                rhs=xT16[:, t0 * B + q * 128:t0 * B + (q + 1) * 128],
                start=True, stop=True, skip_group_check=True,
            )
            if q == 3:
                rt0 = t0 % ZR
                zsl = zv[:, rt0:rt0 + 8, m, :]
                xst = xst_pool.tile([128, 8 * B], F16, name="xst", tag="xst")
                nc.scalar.copy(out=xst[:, :], in_=zps[:, :])
                eng = nc.vector if (w < 2 and m % 2) else nc.gpsimd
                eng.tensor_tensor(
                    out=zsl, in0=zsl,
                    in1=xst[:, :].rearrange("p (t b) -> p t b", b=B),
                    op=AOP.add)
                del zmm_state[(w, c8, m)]

        # prefill: noise windows 0..2, x-projection for windows 0..1
        for w in range(3):
            emit_noise_dma(w)
        for w in range(2):
            for idx in range(WIN * 2):
                emit_zmm_piece(w, idx)

        with tc.tile_pool(name="sp", bufs=2) as sp, \
             tc.tile_pool(name="psA0", bufs=1, space="PSUM") as ps_a0, \
             tc.tile_pool(name="psA1", bufs=1, space="PSUM") as ps_a1, \
             tc.tile_pool(name="psC0", bufs=1, space="PSUM") as ps_c0, \
             tc.tile_pool(name="psC1", bufs=1, space="PSUM") as ps_c1, \
             tc.tile_pool(name="psW", bufs=1, space="PSUM") as ps_w:
            # HAM-warming scratch: filler matmuls keep the PE array busy
            # through the DVE-latency bubble so the clock stays at 2.4 GHz.
            # Alternate 2 disjoint regions so fillers don't WAW-serialize.
            psWarm = ps_w.tile([128, 512], F32, name="pswarm0", tag="psw0")
            warm_ctr = [0]

            def warm_mm(n):
                i = warm_ctr[0]
                warm_ctr[0] += 1
                tgt = psWarm[:8, (i % 2) * 256:(i % 2) * 256 + n]
                pe_mm(tgt, lhsT=zero16[:, 0:8],
                      rhs=xT16[:, 0:n], start=True, stop=True,
                      skip_group_check=True)
            psAs = [ps_a0.tile([128, 512], F32, name="psa0", tag="psa0"),
                    ps_a1.tile([128, 512], F32, name="psa1", tag="psa1")]
            psCs = [ps_c0.tile([128, 512], F32, name="psc0", tag="psc0"),
                    ps_c1.tile([128, 512], F32, name="psc1", tag="psc1")]
            pvAs = [p[:, :].rearrange("p (m c) -> p m c", c=128) for p in psAs]
            pvCs = [p[:, :].rearrange("p (m c) -> p m c", c=128) for p in psCs]

            def ps_of(m, par):
                ps = psAs[par] if m < 2 else psCs[par]
                return ps, (m % 2) * 128

            # prime block-0 banks with zeros
            for m in range(RC):
                ps, off = ps_of(m, 0)
                pe_mm(ps[:, off:off + B], lhsT=ident16[:, :],
                      rhs=zero16[:, :], start=(m % 2 == 0), stop=True,
                      skip_group_check=True)

            for tl in range(S):
                j = _J_OF[tl]
                par_new = _BLK_OF[tl] % 2
                w = tl // WIN
                ph = tl % WIN
                if ph == 0 and 1 <= w and w + 2 <= (S // WIN) - 1:
                    emit_noise_dma(w + 2)

                rd = ((tl - 1) % RU) * SUP
                wr = (tl % RU) * SUP

                if tl > 0:
                    par_kmm = _BLK_OF[tl - 1] % 2

                    def kmm(m, k, stop=False):
                        ps, off = ps_of(m, par_kmm)
                        return pe_mm(
                            ps[:, off:off + B],
                            lhsT=w16[:, k * R + m * 128:k * R + (m + 1) * 128],
                            rhs=rbuf[:, rd + k * B:rd + (k + 1) * B],
                            start=False, stop=stop, skip_group_check=True,
                        )

                    bdry = (j == 0)
                    if bdry:
                        s16a = sp.tile([128, 2 * B], F16, tag="s16a")
                        s16b = sp.tile([128, 2 * B], F16, tag="s16b")
                        scb = float(LEAK ** _LPREV[tl])
                    # A-bank (m01) writes first so ra_{t+1} unblocks ASAP,
                    # then C-bank (m23); k ascending within each.
                    for k in range(4):
                        kmm(0, k, stop=(bdry and k == 3))
                        kmm(1, k, stop=(bdry and k == 3))
                    if bdry:
                        # seed-mul for A half as soon as A banks are final
                        nc.scalar.mul(
                            out=s16a[:, :].rearrange("p (m c) -> p m c", c=B),
                            in_=pvAs[par_kmm][:, 0:2, 0:B], mul=scb)
                    for k in range(4):
                        kmm(2, k, stop=(bdry and k == 3))
                        kmm(3, k, stop=(bdry and k == 3))
                    if bdry:
                        # C-half seed-mul on DVE, parallel with ACT's A-half
                        nc.vector.tensor_scalar_mul(
                            out=s16b[:, :].rearrange("p (m c) -> p m c", c=B),
                            in0=pvCs[par_kmm][:, 0:2, 0:B], scalar1=scb)
                        for m in range(RC):
                            ps, off = ps_of(m, par_new)
                            srct = s16a if m < 2 else s16b
                            pe_mm(
                                ps[:, off:off + B], lhsT=ident16[:, :],
                                rhs=srct[:, (m % 2) * B:(m % 2 + 1) * B],
                                start=(m % 2 == 0), stop=True,
                                skip_group_check=True)

                # zmm pieces (2 per step) for window w+2
                if w + 2 <= (S // WIN) - 1:
                    for pc in (2 * ph, 2 * ph + 1):
                        if pc < WIN * 2:
                            emit_zmm_piece(w + 2, pc)

                # PE filler in the DVE-latency bubble (HAM stays warm)
                warm_mm(96)

                # DVE relu: r~ = relu((A + Z~) * 1.25) -> rbuf ring
                rt = tl % ZR
                nc.vector._custom_dve(
                    RELU_ADD_SC,
                    out=rbuf[:, wr:wr + 2 * B].rearrange(
                        "p (k c) -> p k c", c=B),
                    in0=pvAs[par_new][:, 0:2, 0:B], in1=zv[:, rt, 0:2, :],
                    s0=1.25)
                nc.vector._custom_dve(
                    RELU_ADD_SC,
                    out=rbuf[:, wr + 2 * B:wr + SUP].rearrange(
                        "p (k c) -> p k c", c=B),
                    in0=pvCs[par_new][:, 0:2, 0:B], in1=zv[:, rt, 2:4, :],
                    s0=1.25)

                # dump r~ window every 32 steps
                if ph == WIN - 1:
                    rs = ((w * WIN) % RU) * SUP
                    nc.gpsimd.dma_start(
                        out=o_d[:, w * WIN * SUP:(w + 1) * WIN * SUP],
                        in_=rbuf[:, rs:rs + WIN * SUP],
                    )

    if use_bacc:
        nc.compile()
    return nc


def host_prep(x, w_in, w_rec, b_rec, ei_mask, autapse_mask, noise):
    """Pure marshalling: scale/cast/transpose inputs into per-core images."""
    ei = np.diagonal(np.asarray(ei_mask)).astype(np.float32)
    w_eff = ei[:, None] * (np.asarray(w_rec) * np.asarray(autapse_mask))
    wp = (ALPHA * w_eff).astype(np.float32)
    # w16 image: [p, k*512 + m*128 + c] = W'[k*128+p, m*128+c]
    w_img = np.ascontiguousarray(
        wp.reshape(RC, 128, RC, 128).transpose(1, 0, 2, 3)
        .reshape(128, RC * R)).astype(np.float16)
    wi_img = np.asarray(w_in).astype(np.float16)

    x = np.asarray(x, dtype=np.float32)
    noise = np.asarray(noise, dtype=np.float32)
    b_rec = np.asarray(b_rec, dtype=np.float32)
    T = x.shape[1]
    sc = (ALPHA * (1.0 / LEAK) ** np.array(_J_OF)).astype(np.float32)

    in_maps = []
    for c in range(N_CORES):
        g0 = 125 * c - WU_OFF
        lo, hi = max(0, g0), min(T, g0 + S)
        sl = slice(lo - g0, hi - g0)  # valid local steps
        # x image [128, S*64]: [p, tl*64+b]
        xi = np.zeros((S, B, NIN), np.float32)
        xi[sl] = x[:, lo:hi].transpose(1, 0, 2) * sc[sl, None, None]
        x_img = np.ascontiguousarray(
            xi.transpose(2, 0, 1).reshape(NIN, S * B)).astype(np.float16)
        # noise image [128, S*256]: [p, tl*256 + k*64 + b]
        ni = np.zeros((S, RC, B, 128), np.float32)
        nt = (noise[:, lo:hi] + b_rec).transpose(1, 0, 2)  # (t, b, r)
        ni[sl] = nt.reshape(hi - lo, B, RC, 128).transpose(0, 2, 1, 3) \
            * sc[sl, None, None, None]
        n_img = np.ascontiguousarray(
            ni.transpose(3, 0, 1, 2).reshape(128, S * SUP)).astype(np.float16)
        in_maps.append({
            "x_img": x_img,
            "noise_img": n_img,
            "w16_img": w_img,
            "win16_img": wi_img,
        })
    return in_maps, w_eff.astype(np.float32)


def host_post(results, T):
    """Decode per-core r~ images -> h via per-block cumsum -> (B, T, R) f32."""
    out = np.empty((B, T, R), np.float32)
    for c, res in enumerate(results):
        img = np.asarray(res["out_img"])  # [128, S*256] fp16
        rt = img.reshape(128, S, RC, B).transpose(3, 1, 2, 0) \
            .reshape(B, S, R).astype(np.float32)
        h = np.empty((B, S, R), np.float32)
        hprev = np.zeros((B, R), np.float32)
        s0 = 0
        for L in BLOCKS:
            seg = rt[:, s0:s0 + L]
            desc = (LEAK ** (np.arange(L) + 1.0)).astype(np.float32)
            H = hprev[:, None, :] + np.cumsum(seg, axis=1)
            h[:, s0:s0 + L] = H * desc[None, :, None]
            hprev = h[:, s0 + L - 1]
            s0 += L
        out[:, 125 * c:125 * c + 125] = h[:, WU_OFF:WU_OFF + 125]
    return out


_NC_CACHE = {}


def kernel(x, w_in, w_rec, b_rec, ei_mask, autapse_mask, noise):
    from concourse.bass_utils import run_bass_kernel_spmd

    x = np.asarray(x)
    T = x.shape[1]
    in_maps, _ = host_prep(x, w_in, w_rec, b_rec, ei_mask, autapse_mask, noise)
    if T not in _NC_CACHE:
        _NC_CACHE[T] = build_nc(T=T)
    nc = _NC_CACHE[T]
    res = run_bass_kernel_spmd(nc, in_maps, core_ids=list(range(N_CORES)))
    return host_post(res.results, T)


# revision 20
# speedup vs baseline: 1.0131x; 1.0131x over previous
"""BioRNN Trainium2 kernel — time-sharded, scaled-basis recurrence.

Sharding: 8 cores split T=1000 into 125-step output slices; each core runs
the FULL batch (64) for S=192 local steps: 67 warmup steps (leak 0.8 forgets
the h=0 init; measured truncation err ~7e-4) + 125 output steps.

Per-core math in a scaled basis (J=32 step blocks, j = t mod J,
W' = 0.2*w_eff, z' = 0.2*(x @ w_in + noise + b_rec)):
    A_j  = p_t / 0.8^j       PSUM f32, accumulate-only: A += r~_{j-1} @ W'
    r~_j = r_t / 0.8^(j+1)   = relu((A_j + Z~_j) * 1.25)   DVE -> fp16 ring
    Z~_j = z'_t / 0.8^j      zbuf (noise part DMA'd pre-scaled, x@w_in added
                             on-device via PE + Pool)
Block boundary: A'_0 = 0.8^J * A_J via ACT mul -> fp16 -> identity-matmul
seed into the opposite PSUM bank pair.  No per-step decay ops remain: a step
is 16 LDW+MM pairs (N=64) + 2 DVE relu ops.  h_t itself is reconstructed on
the host from the dumped r~ stream by a per-block f32 cumsum:
    h_t = 0.8^(j+1) * (h_{blockstart-1} + sum_{i<=j} r~_i).

Host does pure marshalling/cumsum: inputs pre-transposed to hidden-major
fp16 images with the 0.2*0.8^(-j) scale baked in; output is the raw fp16
r~-image.

Layouts (per core, hidden chunk k = r//128, partition p = r%128):
  w16   [128, k*512 + m*128 + c] = W'[k*128+p, m*128+c]      fp16
  win16 [128, 512]   (n_in on partitions)                    fp16
  xT16  [128, tl*64 + b]      = x[b, g0+tl, p] * sc(tl)      fp16
  zbuf  [128, (tl%ZR)*256 + k*64 + b]  ring                  fp16
  rbuf  [128, (tl%RU)*256 + k*64 + b]  ring (r~)             fp16
  out   [128, tl*256 + k*64 + b]  (raw r~ dump, all S steps) fp16
"""

import numpy as np
from contextlib import ExitStack

import concourse.bass as bass
import concourse.mybir as mybir
import concourse.tile as tile
from concourse import bacc
from concourse import dve_ops
from concourse.dve_spec import Spec, Src0, Src1, C0, relu as _dve_relu_expr, lower
from concourse.dve_uop import DveOpSpec
from concourse.masks import make_identity


def _register_relu_add_sc():
    """Register fused out = relu((in0 + in1) * s0) custom DVE op (idempotent)."""
    name = "RELU_ADD_SC_BIO"
    for o in dve_ops.OPS:
        if o.name == name:
            return o
    opcode = max(dve_ops._SUB_OPCODE_FOR_NAME.values()) + 1
    assert opcode < 0x20
    dve_ops._SUB_OPCODE_FOR_NAME[name] = opcode

    def _ref(in0, in1, c0, c1, c2):
        a = in0.astype(np.float32).reshape(in0.shape[0], -1)
        b = in1.astype(np.float32).reshape(in1.shape[0], -1)
        s = np.maximum(np.nan_to_num((a + b) * c0, nan=0.0, posinf=np.inf,
                                     neginf=-np.inf), 0)
        return s.reshape(in0.shape)

    spec = Spec(body=_dve_relu_expr((Src0 + Src1) * C0), reference=_ref)
    shas = {}
    for ver in ("v3", "v4"):
        s = DveOpSpec(name=name, opcode=opcode, uops=lower(spec, ver=ver),
                      rd1_en=True)
        shas[ver] = s.sha(ver)
    op = dve_ops.DveOp(name, spec, subdim=False, uops_sha=shas)
    dve_ops.OPS.append(op)
    dve_ops.CUSTOM_DVE_SPECS[name] = spec
    return op


RELU_ADD_SC = _register_relu_add_sc()

F32 = mybir.dt.float32
F16 = mybir.dt.float16
AOP = mybir.AluOpType

B = 64           # batch (full, replicated across cores)
R = 512          # n_rec
NIN = 128        # n_in
RC = 4           # hidden chunks of 128
SUP = RC * B     # 256 cols per step
N_CORES = 8
ALPHA = 0.2
LEAK = 1.0 - ALPHA

S = 176          # local steps per core
WU_OFF = 51      # g0 = 125*c - WU_OFF
RU = 64          # r~ ring steps
ZR = 48          # zbuf ring steps (3 windows)
WIN = 16         # zmm/noise/dump window
BLOCKS = [16, 40, 40, 40, 40]       # rescale block lengths (sum = S)
_J_OF, _BLK_OF, _LPREV = [], [], {}
for _bi, _L in enumerate(BLOCKS):
    _s = len(_J_OF)
    if _bi > 0:
        _LPREV[_s] = BLOCKS[_bi - 1]
    for _j in range(_L):
        _J_OF.append(_j)
        _BLK_OF.append(_bi)
assert len(_J_OF) == S


def build_nc(T=1000, use_bacc=True):
    assert T == 1000
    nc = bacc.Bacc() if use_bacc else bass.Bass()

    x_d = nc.dram_tensor("x_img", [128, S * B], F16, kind="ExternalInput").ap()
    n_d = nc.dram_tensor("noise_img", [128, S * SUP], F16,
                         kind="ExternalInput").ap()
    w_d = nc.dram_tensor("w16_img", [128, RC * R], F16,
                         kind="ExternalInput").ap()
    wi_d = nc.dram_tensor("win16_img", [128, R], F16,
                          kind="ExternalInput").ap()
    o_d = nc.dram_tensor("out_img", [128, S * SUP], F16,
                         kind="ExternalOutput").ap()

    pe_last = [None]

    def pe_mm(*args, **kwargs):
        mm = nc.tensor.matmul(*args, **kwargs)
        if pe_last[0] is not None:
            tile.add_dep_helper(mm.ins, pe_last[0], sync=False,
                                reason="pe program order")
        pe_last[0] = mm.ins
        return mm

    with tile.TileContext(nc) as tc, ExitStack() as ctx:
        const = ctx.enter_context(tc.tile_pool(name="const", bufs=1))
        big = ctx.enter_context(tc.tile_pool(name="big", bufs=1))

        ident16 = const.tile([128, 128], F16)
        make_identity(nc, ident16[:, :])
        zero16 = const.tile([128, B], F16)
        nc.vector.memset(zero16[:, :], 0.0)

        w16 = const.tile([128, RC * R], F16)
        nc.sync.dma_start(out=w16[:, :], in_=w_d)
        win16 = const.tile([128, R], F16)
        nc.sync.dma_start(out=win16[:, :], in_=wi_d)

        xT16 = big.tile([128, S * B], F16)
        for xc in range(6):
            c0, c1 = xc * 2 * WIN * B, min((xc + 1) * 2 * WIN * B, S * B)
            nc.gpsimd.dma_start(out=xT16[:, c0:c1], in_=x_d[:, c0:c1])

        zbuf = big.tile([128, ZR * SUP], F16)
        rbuf = big.tile([128, RU * SUP], F16)

        zv = zbuf[:, :].rearrange("p (t k b) -> p t k b", k=RC, b=B)

        ps_z = ctx.enter_context(tc.tile_pool(name="psz", bufs=3, space="PSUM"))
        xst_pool = ctx.enter_context(tc.tile_pool(name="xst", bufs=4))

        def emit_noise_dma(w):
            rt0 = (w * WIN) % ZR
            nc.sync.dma_start(
                out=zbuf[:, rt0 * SUP:(rt0 + WIN) * SUP],
                in_=n_d[:, w * WIN * SUP:(w + 1) * WIN * SUP],
            )

        # zmm piece: one (8-step chunk, m, quarter) 128-col matmul; after the
        # 4th quarter, a Pool TT-add folds psum into zbuf (noise already there).
        zmm_state = {}

        def emit_zmm_piece(w, idx):
            c8, m, q = idx // 16, (idx // 4) % 4, idx % 4
            assert c8 < WIN // 8
            t0 = w * WIN + c8 * 8
            if q == 0:
                zmm_state[(w, c8, m)] = ps_z.tile([128, 8 * B], F32,
                                                  name="zps", tag="zps")
            zps = zmm_state[(w, c8, m)]
            pe_mm(
                zps[:, q * 128:(q + 1) * 128],
                lhsT=win16[:, m * 128:(m + 1) * 128],
                rhs=xT16[:, t0 * B + q * 128:t0 * B + (q + 1) * 128],
                start=True, stop=True, skip_group_check=True,
            )
            if q == 3:
                rt0 = t0 % ZR
                zsl = zv[:, rt0:rt0 + 8, m, :]
                xst = xst_pool.tile([128, 8 * B], F16, name="xst", tag="xst")
                nc.scalar.copy(out=xst[:, :], in_=zps[:, :])
                eng = nc.vector if (w < 2 and m % 2) else nc.gpsimd
                eng.tensor_tensor(
                    out=zsl, in0=zsl,
                    in1=xst[:, :].rearrange("p (t b) -> p t b", b=B),
                    op=AOP.add)
                del zmm_state[(w, c8, m)]

        # prefill: noise windows 0..2, x-projection for window 0 only
        # (window 1 pieces are folded into steps 0..15)
        for w in range(3):
            emit_noise_dma(w)
        for idx in range(WIN * 2):
            emit_zmm_piece(0, idx)

        with tc.tile_pool(name="sp", bufs=2) as sp, \
             tc.tile_pool(name="psA0", bufs=1, space="PSUM") as ps_a0, \
             tc.tile_pool(name="psA1", bufs=1, space="PSUM") as ps_a1, \
             tc.tile_pool(name="psC0", bufs=1, space="PSUM") as ps_c0, \
             tc.tile_pool(name="psC1", bufs=1, space="PSUM") as ps_c1, \
             tc.tile_pool(name="psW", bufs=1, space="PSUM") as ps_w:
            # HAM-warming scratch: filler matmuls keep the PE array busy
            # through the DVE-latency bubble so the clock stays at 2.4 GHz.
            # Alternate 2 disjoint regions so fillers don't WAW-serialize.
            psWarm = ps_w.tile([128, 512], F32, name="pswarm0", tag="psw0")
            warm_ctr = [0]

            def warm_mm(n):
                i = warm_ctr[0]
                warm_ctr[0] += 1
                tgt = psWarm[:8, (i % 2) * 256:(i % 2) * 256 + n]
                pe_mm(tgt, lhsT=zero16[:, 0:8],
                      rhs=xT16[:, 0:n], start=True, stop=True,
                      skip_group_check=True)
            psAs = [ps_a0.tile([128, 512], F32, name="psa0", tag="psa0"),
                    ps_a1.tile([128, 512], F32, name="psa1", tag="psa1")]
            psCs = [ps_c0.tile([128, 512], F32, name="psc0", tag="psc0"),
                    ps_c1.tile([128, 512], F32, name="psc1", tag="psc1")]
            pvAs = [p[:, :].rearrange("p (m c) -> p m c", c=128) for p in psAs]
            pvCs = [p[:, :].rearrange("p (m c) -> p m c", c=128) for p in psCs]

            def ps_of(m, par):
                ps = psAs[par] if m < 2 else psCs[par]
                return ps, (m % 2) * 128

            # prime block-0 banks with zeros
            for m in range(RC):
                ps, off = ps_of(m, 0)
                pe_mm(ps[:, off:off + B], lhsT=ident16[:, :],
                      rhs=zero16[:, :], start=(m % 2 == 0), stop=True,
                      skip_group_check=True)

            for tl in range(S):
                j = _J_OF[tl]
                par_new = _BLK_OF[tl] % 2
                w = tl // WIN
                ph = tl % WIN
                if ph == 0 and 1 <= w and w + 2 <= (S // WIN) - 1:
                    emit_noise_dma(w + 2)

                rd = ((tl - 1) % RU) * SUP
                wr = (tl % RU) * SUP

                if tl > 0:
                    par_kmm = _BLK_OF[tl - 1] % 2

                    def kmm(m, k, stop=False):
                        ps, off = ps_of(m, par_kmm)
                        return pe_mm(
                            ps[:, off:off + B],
                            lhsT=w16[:, k * R + m * 128:k * R + (m + 1) * 128],
                            rhs=rbuf[:, rd + k * B:rd + (k + 1) * B],
                            start=False, stop=stop, skip_group_check=True,
                        )

                    bdry = (j == 0)
                    if bdry:
                        s16a = sp.tile([128, 2 * B], F16, tag="s16a")
                        s16b = sp.tile([128, 2 * B], F16, tag="s16b")
                        scb = float(LEAK ** _LPREV[tl])
                    # A-bank (m01) writes first so ra_{t+1} unblocks ASAP,
                    # then C-bank (m23); k ascending within each.
                    for k in range(4):
                        kmm(0, k, stop=(bdry and k == 3))
                        kmm(1, k, stop=(bdry and k == 3))
                    if bdry:
                        # seed-mul for A half as soon as A banks are final
                        nc.scalar.mul(
                            out=s16a[:, :].rearrange("p (m c) -> p m c", c=B),
                            in_=pvAs[par_kmm][:, 0:2, 0:B], mul=scb)
                    for k in range(4):
                        kmm(2, k, stop=(bdry and k == 3))
                        kmm(3, k, stop=(bdry and k == 3))
                    if bdry:
                        # C-half seed-mul on DVE, parallel with ACT's A-half
                        nc.vector.tensor_scalar_mul(
                            out=s16b[:, :].rearrange("p (m c) -> p m c", c=B),
                            in0=pvCs[par_kmm][:, 0:2, 0:B], scalar1=scb)
                        for m in range(RC):
                            ps, off = ps_of(m, par_new)
                            srct = s16a if m < 2 else s16b
                            pe_mm(
                                ps[:, off:off + B], lhsT=ident16[:, :],
                                rhs=srct[:, (m % 2) * B:(m % 2 + 1) * B],
                                start=(m % 2 == 0), stop=True,
                                skip_group_check=True)

                # zmm pieces (2 per step) for window w+2 (+ w1 early)
                if tl < WIN:
                    for pc in (2 * ph, 2 * ph + 1):
                        emit_zmm_piece(1, pc)
                if w + 2 <= (S // WIN) - 1:
                    for pc in (2 * ph, 2 * ph + 1):
                        if pc < WIN * 2:
                            emit_zmm_piece(w + 2, pc)

                # PE filler in the DVE-latency bubble (HAM stays warm)
                warm_mm(96)

                # DVE relu: r~ = relu((A + Z~) * 1.25) -> rbuf ring
                rt = tl % ZR
                nc.vector._custom_dve(
                    RELU_ADD_SC,
                    out=rbuf[:, wr:wr + 2 * B].rearrange(
                        "p (k c) -> p k c", c=B),
                    in0=pvAs[par_new][:, 0:2, 0:B], in1=zv[:, rt, 0:2, :],
                    s0=1.25)
                nc.vector._custom_dve(
                    RELU_ADD_SC,
                    out=rbuf[:, wr + 2 * B:wr + SUP].rearrange(
                        "p (k c) -> p k c", c=B),
                    in0=pvCs[par_new][:, 0:2, 0:B], in1=zv[:, rt, 2:4, :],
                    s0=1.25)

                # dump r~ window every 32 steps
                if ph == WIN - 1:
                    rs = ((w * WIN) % RU) * SUP
                    nc.gpsimd.dma_start(
                        out=o_d[:, w * WIN * SUP:(w + 1) * WIN * SUP],
                        in_=rbuf[:, rs:rs + WIN * SUP],
                    )

    if use_bacc:
        nc.compile()
    return nc


def host_prep(x, w_in, w_rec, b_rec, ei_mask, autapse_mask, noise):
    """Pure marshalling: scale/cast/transpose inputs into per-core images."""
    ei = np.diagonal(np.asarray(ei_mask)).astype(np.float32)
    w_eff = ei[:, None] * (np.asarray(w_rec) * np.asarray(autapse_mask))
    wp = (ALPHA * w_eff).astype(np.float32)
    # w16 image: [p, k*512 + m*128 + c] = W'[k*128+p, m*128+c]
    w_img = np.ascontiguousarray(
        wp.reshape(RC, 128, RC, 128).transpose(1, 0, 2, 3)
        .reshape(128, RC * R)).astype(np.float16)
    wi_img = np.asarray(w_in).astype(np.float16)

    x = np.asarray(x, dtype=np.float32)
    noise = np.asarray(noise, dtype=np.float32)
    b_rec = np.asarray(b_rec, dtype=np.float32)
    T = x.shape[1]
    sc = (ALPHA * (1.0 / LEAK) ** np.array(_J_OF)).astype(np.float32)

    in_maps = []
    for c in range(N_CORES):
        g0 = 125 * c - WU_OFF
        lo, hi = max(0, g0), min(T, g0 + S)
        sl = slice(lo - g0, hi - g0)  # valid local steps
        # x image [128, S*64]: [p, tl*64+b]
        xi = np.zeros((S, B, NIN), np.float32)
        xi[sl] = x[:, lo:hi].transpose(1, 0, 2) * sc[sl, None, None]
        x_img = np.ascontiguousarray(
            xi.transpose(2, 0, 1).reshape(NIN, S * B)).astype(np.float16)
        # noise image [128, S*256]: [p, tl*256 + k*64 + b]
        ni = np.zeros((S, RC, B, 128), np.float32)
        nt = (noise[:, lo:hi] + b_rec).transpose(1, 0, 2)  # (t, b, r)
        ni[sl] = nt.reshape(hi - lo, B, RC, 128).transpose(0, 2, 1, 3) \
            * sc[sl, None, None, None]
        n_img = np.ascontiguousarray(
            ni.transpose(3, 0, 1, 2).reshape(128, S * SUP)).astype(np.float16)
        in_maps.append({
            "x_img": x_img,
            "noise_img": n_img,
            "w16_img": w_img,
            "win16_img": wi_img,
        })
    return in_maps, w_eff.astype(np.float32)


def host_post(results, T):
    """Decode per-core r~ images -> h via per-block cumsum -> (B, T, R) f32."""
    out = np.empty((B, T, R), np.float32)
    for c, res in enumerate(results):
        img = np.asarray(res["out_img"])  # [128, S*256] fp16
        rt = img.reshape(128, S, RC, B).transpose(3, 1, 2, 0) \
            .reshape(B, S, R).astype(np.float32)
        h = np.empty((B, S, R), np.float32)
        hprev = np.zeros((B, R), np.float32)
        s0 = 0
        for L in BLOCKS:
            seg = rt[:, s0:s0 + L]
            desc = (LEAK ** (np.arange(L) + 1.0)).astype(np.float32)
            H = hprev[:, None, :] + np.cumsum(seg, axis=1)
            h[:, s0:s0 + L] = H * desc[None, :, None]
            hprev = h[:, s0 + L - 1]
            s0 += L
        out[:, 125 * c:125 * c + 125] = h[:, WU_OFF:WU_OFF + 125]
    return out


_NC_CACHE = {}


def kernel(x, w_in, w_rec, b_rec, ei_mask, autapse_mask, noise):
    from concourse.bass_utils import run_bass_kernel_spmd

    x = np.asarray(x)
    T = x.shape[1]
    in_maps, _ = host_prep(x, w_in, w_rec, b_rec, ei_mask, autapse_mask, noise)
    if T not in _NC_CACHE:
        _NC_CACHE[T] = build_nc(T=T)
    nc = _NC_CACHE[T]
    res = run_bass_kernel_spmd(nc, in_maps, core_ids=list(range(N_CORES)))
    return host_post(res.results, T)
